# revision 26
# baseline (speedup 1.0000x reference)
"""Trainium2 Bass kernel for nn_Attention_51539608408.

Math note: the reference applies softmax over an axis of size 1, which is
identically 1.0. Consequently the outputs depend only on `a` and `X_mask`:
    alpha[b, t] = mask[b, t] / count[b]          (count = sum_t mask)
    context[b, 0, :] = sum_t mask[b, t] * a[b, t, :] / count[b]
All the Wa/Wh/Wc/V matmuls and tanh are dead compute (verified exactly: the
reference's jax.nn.softmax subtracts the max, so exp(0)/1 == 1.0 exactly).

Sharding: pure data-parallel over batch B=32 -> 4 examples per core x 8 cores.

Per-core device kernel (per example b):
  - t axis (4096) split partition-major: t = p*32 + n  (p in [0,128), n in [0,32))
    so each SBUF partition holds a contiguous 64KB slab of DRAM -> fast DMA.
  - masked sum over t via PE matmul: lhsT = mask column [128,1], rhs = a tile
    [128, 512], accumulated over the 32 sub-tiles into PSUM [1, 512].
  - count via matmul with a ones vector, reciprocal on DVE, broadcast back to
    128 partitions with a tiny [1,128]-ones matmul.
"""

import os
import sys

import numpy as np

for _p in ("/opt/trn_rl_repo", "/opt/trn_rl_repo/concourse"):
    if os.path.isdir(_p) and _p not in sys.path:
        sys.path.insert(0, _p)

from contextlib import ExitStack

from concourse import bacc, bass, mybir, tile
from concourse.bass_utils import run_bass_kernel_spmd

B, TX, D = 32, 4096, 512
NCORES = 8
BPC = B // NCORES  # examples per core
P = 128            # SBUF partitions
NSUB = TX // P     # 32 sub-tiles along t
F32 = mybir.dt.float32
F32R = mybir.dt.float32r
U8 = mybir.dt.uint8


def build_nc_raw(chunk=8, nbuf=8):
    """Hand-scheduled raw-Bass version: no Tile entry/exit barriers.

    Engine roles:
      Sync   — streams the 16 big `a` chunk DMAs back-to-back (HWDGE)
      Scalar — small DMAs: mask in, alpha/ctx out (HWDGE)
      Tensor — 2 batched count matmuls + 4x32 masked-sum f32r matmuls
      Vector — casts, reductions, reciprocal, scaling
      GpSimd — epilogue: restore semaphores to zero for re-executability
    """
    NCHUNK = NSUB // chunk  # chunks per example
    # chunk layout per example: sub-tile counts for each DMA. The final
    # example tapers so the last-DMA -> last-matmul dependency is short.
    chunk_lists = [[chunk] * NCHUNK for _ in range(BPC)]
    chunk_lists[BPC - 1] = [chunk] * (NCHUNK - 1) + [4, 2, 1, 1]
    cum_chunks = []
    tot = 0
    for cl in chunk_lists:
        tot += len(cl)
        cum_chunks.append(tot)
    nc = bass.Bass("TRN2")
    a_d = nc.dram_tensor("a", [BPC, TX, D], F32R, kind="ExternalInput")
    # mask pre-packed on host to [P, BPC*NSUB] so each partition's row is
    # contiguous in DRAM (fast descriptor generation, single DMA)
    m_d = nc.dram_tensor("m", [P, BPC * NSUB], U8, kind="ExternalInput")
    ctx_d = nc.dram_tensor("ctx", [BPC, D], F32, kind="ExternalOutput")
    alpha_d = nc.dram_tensor("alpha", [BPC, TX], F32, kind="ExternalOutput")

    with ExitStack() as st:
        a_bufs = [st.enter_context(nc.sbuf_tensor(f"a_buf{i}", [P, chunk, D], F32R))
                  for i in range(nbuf)]
        m_u8 = st.enter_context(nc.sbuf_tensor("m_u8", [P, BPC, NSUB], U8))
        m_f = st.enter_context(nc.sbuf_tensor("m_f", [P, BPC, NSUB], F32R))
        alpha_sbs = [st.enter_context(nc.sbuf_tensor(f"alpha_sb{b}", [P, NSUB], F32))
                     for b in range(BPC)]
        ctx_sbs = [st.enter_context(nc.sbuf_tensor(f"ctx_sb{b}", [1, D], F32))
                   for b in range(BPC)]
        ones_col = st.enter_context(nc.sbuf_tensor("ones_col", [P, 1], F32))
        ones_row = st.enter_context(nc.sbuf_tensor("ones_row", [1, P], F32))
        partials = st.enter_context(nc.sbuf_tensor("partials", [P, BPC], F32))
        cnt_all = st.enter_context(nc.sbuf_tensor("cnt_all", [1, BPC], F32))
        invb_all = st.enter_context(nc.sbuf_tensor("invb_all", [P, BPC], F32))
        # one full PSUM bank each so PE writes never share a bank with
        # concurrent DVE reads (same-bank PE-W + DVE-R is a hard fault)
        pbank_ctx = [st.enter_context(nc.psum_tensor(f"pb_ctx{b}", [P, D], F32))
                     for b in range(BPC)]
        pbank_cnt = st.enter_context(nc.psum_tensor("pb_cnt", [P, D], F32))
        pbank_cntb = st.enter_context(nc.psum_tensor("pb_cntb", [P, D], F32))
        warm_buf = st.enter_context(nc.sbuf_tensor("warm_buf", [P, D], F32))
        pbank_warm = st.enter_context(nc.psum_tensor("pb_warm", [P, D], F32))

        NCH_ALL = cum_chunks[-1]
        with (
            nc.Block() as block,
            nc.semaphore("sem_m") as sem_m,
            nc.semaphore("sem_dve") as sem_dve,
            nc.semaphore("sem_pe") as sem_pe,
            nc.semaphore("sem_free") as sem_free,
            nc.semaphore("sem_out") as sem_out,
            nc.semaphore("sem_warm") as sem_warm,
        ):
            # one semaphore per chunk DMA: HWDGE transfers from one engine can
            # complete out of order across HW queues, so cumulative waits on a
            # single sem are unsound (CoreSim SemaphoreRace).
            sem_a = [st.enter_context(nc.semaphore(f"sem_a{i}"))
                     for i in range(NCH_ALL)]
            sems = [sem_m, sem_dve, sem_pe, sem_free, sem_out, sem_warm] + sem_a
            # DVE tick numbering (sem_dve): cast=1, partials=2..5, cnt=6,
            # recip=7, alpha=8..11, then ctx combines. The last example
            # accumulates into a single PSUM bank so its combine is one op.
            ALPHA_TICK = [8 + b for b in range(BPC)]
            CTX_TICK = []
            t = ALPHA_TICK[-1]
            for b in range(BPC):
                t += 1 if b == BPC - 1 else 2
                CTX_TICK.append(t)

            @block.sync
            def _(sync):
                sync.dma_start(m_u8[:], m_d[:]).then_inc(sem_m, 16)
                i = 0
                for b in range(BPC):
                    a_re = a_d[b].rearrange("(p n) d -> p n d", p=P)
                    off = 0
                    for w in chunk_lists[b]:
                        if i >= nbuf:
                            sync.wait_ge(sem_free, i - nbuf + 1)
                        sync.dma_start(
                            a_bufs[i % nbuf][:, 0:w, :],
                            a_re[:, off:off + w, :],
                        ).then_inc(sem_a[i], 16)
                        off += w
                        i += 1
                for b in range(BPC):
                    sync.wait_ge(sem_dve, CTX_TICK[b])
                    sync.dma_start(ctx_d[b], ctx_sbs[b][0:1, :]).then_inc(
                        sem_out, 16)



            @block.scalar
            def _(scalar):
                pass  # barrier legs only; all small DMAs ride SWDGE on gpsimd

            @block.vector
            def _(vector):
                nc.vector.memset(warm_buf[:], 1.0).then_inc(sem_warm)
                nc.vector.memset(ones_col[:], 1.0)
                nc.vector.memset(ones_row[:], 1.0)
                vector.wait_ge(sem_m, 16)
                nc.vector.tensor_copy(m_f[:], m_u8[:]).then_inc(sem_dve)  # 1
                vector.wait_ge(sem_dve, 1)  # same-engine RAW on m_f
                for b in range(BPC):
                    nc.vector.reduce_sum(
                        partials[:, b:b + 1], m_f[:, b, :].bitcast(F32),
                        axis=mybir.AxisListType.X,
                    ).then_inc(sem_dve)  # 2+b
                vector.wait_ge(sem_pe, 1)
                nc.vector.tensor_copy(
                    cnt_all[:], pbank_cnt[0:1, 0:BPC]).then_inc(sem_dve)  # 6
                vector.wait_ge(sem_pe, 2)
                nc.vector.reciprocal(
                    invb_all[:], pbank_cntb[:, 0:BPC]).then_inc(sem_dve)  # 7
                vector.wait_ge(sem_dve, 7)  # same-engine RAW on invb_all
                for b in range(BPC):
                    nc.vector.tensor_scalar_mul(
                        alpha_sbs[b][:], m_f[:, b, :].bitcast(F32),
                        invb_all[:, b:b + 1],
                    ).then_inc(sem_dve)  # 8+b
                for b in range(BPC):
                    bankA = pbank_ctx[(b % 2) * 2]
                    vector.wait_ge(sem_free, cum_chunks[b])
                    nc.vector.tensor_scalar_mul(
                        ctx_sbs[b][0:1, :], bankA[0:1, :],
                        invb_all[0:1, b:b + 1],
                    ).then_inc(sem_dve)
                    if b < BPC - 1:
                        bankB = pbank_ctx[(b % 2) * 2 + 1]
                        vector.wait_ge(sem_dve, CTX_TICK[b] - 1)  # same-engine RAW
                        # ctx = (bankB * inv) + ctx  (one PSUM input per DVE op)
                        nc.vector.scalar_tensor_tensor(
                            ctx_sbs[b][0:1, :], bankB[0:1, :],
                            invb_all[0:1, b:b + 1], ctx_sbs[b][0:1, :],
                            op0=mybir.AluOpType.mult, op1=mybir.AluOpType.add,
                        ).then_inc(sem_dve)

            @block.tensor
            def _(tensor):
                # warm the PE HAM window while the first chunks stream in
                tensor.wait_ge(sem_warm, 1)
                for _ in range(3):
                    nc.tensor.matmul(pbank_warm[0:P, :], warm_buf[:, 0:P],
                                     warm_buf[:], start=True, stop=True)

                def chain(tensor, b, i0):
                    # alternate between two PSUM banks so consecutive matmuls
                    # overlap drain with fill (~427ns/mm instead of ~628ns);
                    # the last example uses one bank so the end-of-kernel
                    # combine is a single DVE op
                    dual = b < BPC - 1
                    bankA = pbank_ctx[(b % 2) * 2]
                    bankB = pbank_ctx[(b % 2) * 2 + 1] if dual else bankA
                    i = i0
                    n = 0
                    for w in chunk_lists[b]:
                        tensor.wait_ge(sem_a[i], 16)
                        for j in range(w):
                            mm = nc.tensor.matmul(
                                (bankA if n % 2 == 0 else bankB)[0:1, :],
                                m_f[:, b, n:n + 1],
                                a_bufs[i % nbuf][:, j, :],
                                start=(n <= 1 if dual else n == 0),
                                stop=(n >= NSUB - 2 if dual else n == NSUB - 1),
                            )
                            if j == w - 1:
                                mm.then_inc(sem_free)
                            n += 1
                        i += 1
                    return i

                tensor.wait_ge(sem_dve, 1)  # mask cast done
                i = chain(tensor, 0, 0)
                # counts: after chain 0 so they never delay the pipeline start
                tensor.wait_ge(sem_dve, 1 + BPC)
                nc.tensor.matmul(pbank_cnt[0:1, 0:BPC], ones_col[:], partials[:],
                                 start=True, stop=True).then_inc(sem_pe)
                tensor.wait_ge(sem_dve, 2 + BPC)
                nc.tensor.matmul(pbank_cntb[:, 0:BPC], ones_row[:], cnt_all[:],
                                 start=True, stop=True).then_inc(sem_pe)
                for b in range(1, BPC):
                    if b >= 2:
                        # WAR: chain(b) reuses chain(b-2)'s banks; wait for the
                        # DVE combine of example b-2 to have read BOTH banks
                        tensor.wait_ge(sem_dve, CTX_TICK[b - 2])
                    i = chain(tensor, b, i)

            @block.gpsimd
            def _(gpsimd):
                # outputs ride SWDGE rings, fully separate from the HWDGE
                # queue streaming the 2MB chunk loads
                for b in range(BPC):
                    gpsimd.wait_ge(sem_dve, 8 + b)
                    gpsimd.dma_start(
                        alpha_d[b].rearrange("(p n) -> p n", p=P), alpha_sbs[b][:]
                    ).then_inc(sem_out, 16)
                # hold kernel end until the output DMAs have landed; the
                # all-engine barrier below covers every other semaphore
                gpsimd.wait_ge(sem_out, 16 * 2 * BPC)

            # one all-engine barrier, then restore sems to zero so the NEFF
            # stays correct if the loaded executable is run a second time
            nc.multi_engine_barrier(list(nc.engines))
            nums = sorted(s.num for s in sems)
            assert nums == list(range(nums[0], nums[-1] + 1)), nums
            nc.gpsimd.dma_reset(range(nums[0], nums[-1] + 1))
            nc.gpsimd.sem_clear(range(nums[0], nums[-1] + 1))

    return nc


def build_nc(variant="f32r", chunk=8, debug=False):
    """Build the per-core Bass program.

    variant: 'f32r' (PE matmul, float32r), 'f32' (PE matmul, float32),
             'dve'  (VectorE masked accumulate + one fp32 matmul reduce)
    """
    assert NSUB % chunk == 0
    a_dt = {"f32r": F32R, "f32": F32, "dve": F32}[variant]

    nc = bacc.Bacc("TRN2", target_bir_lowering=False, debug=debug)
    a_d = nc.dram_tensor("a", [BPC, TX, D], a_dt, kind="ExternalInput")
    m_d = nc.dram_tensor("m", [BPC, TX], U8, kind="ExternalInput")
    ctx_d = nc.dram_tensor("ctx", [BPC, D], F32, kind="ExternalOutput")
    alpha_d = nc.dram_tensor("alpha", [BPC, TX], F32, kind="ExternalOutput")

    with tile.TileContext(nc) as tc, ExitStack() as ctx:
        apool = ctx.enter_context(tc.tile_pool(name="a", bufs=5))
        mpool = ctx.enter_context(tc.tile_pool(name="m", bufs=BPC))
        aux = ctx.enter_context(tc.tile_pool(name="aux", bufs=BPC))
        outp = ctx.enter_context(tc.tile_pool(name="out", bufs=2))
        const = ctx.enter_context(tc.tile_pool(name="const", bufs=1))
        pctx = ctx.enter_context(tc.tile_pool(name="pctx", bufs=2, space="PSUM"))
        pcnt = ctx.enter_context(tc.tile_pool(name="pcnt", bufs=2, space="PSUM"))

        ones_col = const.tile([P, 1], F32, tag="ones_col")
        nc.vector.memset(ones_col[:], 1.0)
        ones_row = const.tile([1, P], F32, tag="ones_row")
        nc.vector.memset(ones_row[:], 1.0)

        # Phase 1: everything that depends only on the mask, for all examples.
        # Small DMAs ride the Scalar-engine HWDGE queue so the Sync-engine
        # queue streams the 16 big `a` chunk DMAs with no interleaved waits.
        m_fs, invbs = [], []
        for b in range(BPC):
            m_u8 = mpool.tile([P, NSUB], U8, tag="mu8")
            nc.scalar.dma_start(m_u8[:], m_d[b].rearrange("(p n) -> p n", p=P))
            m_f = mpool.tile([P, NSUB], a_dt, tag="mf")
            nc.vector.tensor_copy(m_f[:], m_u8[:])
            m_fs.append(m_f)

            # count and its reciprocal, broadcast to all partitions
            partials = aux.tile([P, 1], F32, tag="partials")
            nc.vector.reduce_sum(partials[:], m_f[:].bitcast(F32),
                                 axis=mybir.AxisListType.X)
            cnt1_ps = pcnt.tile([1, 1], F32, tag="cnt1")
            nc.tensor.matmul(cnt1_ps[:], ones_col[:], partials[:],
                             start=True, stop=True)
            cnt = aux.tile([1, 1], F32, tag="cnt_sb")
            nc.vector.tensor_copy(cnt[:], cnt1_ps[:])
            cntb_ps = pcnt.tile([P, 1], F32, tag="cntb")
            nc.tensor.matmul(cntb_ps[:], ones_row[:], cnt[:], start=True, stop=True)
            invb = aux.tile([P, 1], F32, tag="invb")
            nc.vector.reciprocal(invb[:], cntb_ps[:])
            invbs.append(invb)

            # alpha = mask * (1/count)
            alpha_sb = outp.tile([P, NSUB], F32, tag="alpha")
            nc.vector.tensor_scalar_mul(alpha_sb[:], m_f[:].bitcast(F32), invb[:])
            nc.scalar.dma_start(alpha_d[b].rearrange("(p n) -> p n", p=P),
                                alpha_sb[:])

        # Phase 2: context = (1/count) * sum_t mask*a — the memory-bound part.
        for b in range(BPC):
            m_f = m_fs[b]
            invb = invbs[b]
            a_re = a_d[b].rearrange("(p n) d -> p n d", p=P)  # [128, 32, 512]
            ctx_ps = pctx.tile([1, D], F32, tag="ctxps")
            if variant == "dve":
                acc = apool.tile([P, D], F32, tag="acc")
                for c in range(NSUB // chunk):
                    a_t = apool.tile([P, chunk, D], a_dt, tag="achunk")
                    nc.sync.dma_start(a_t[:], a_re[:, c * chunk:(c + 1) * chunk, :])
                    for j in range(chunk):
                        n = c * chunk + j
                        if n == 0:
                            nc.vector.tensor_scalar_mul(
                                acc[:], a_t[:, j, :], m_f[:, 0:1])
                        else:
                            masked = apool.tile([P, D], F32, tag="masked")
                            nc.vector.tensor_scalar_mul(
                                masked[:], a_t[:, j, :], m_f[:, n:n + 1])
                            nc.vector.tensor_add(acc[:], acc[:], masked[:])
                nc.tensor.matmul(ctx_ps[:], ones_col[:], acc[:], start=True, stop=True)
            else:
                for c in range(NSUB // chunk):
                    a_t = apool.tile([P, chunk, D], a_dt, tag="achunk")
                    nc.sync.dma_start(a_t[:], a_re[:, c * chunk:(c + 1) * chunk, :])
                    for j in range(chunk):
                        n = c * chunk + j
                        nc.tensor.matmul(
                            ctx_ps[:],
                            m_f[:, n:n + 1],
                            a_t[:, j, :],
                            start=(n == 0),
                            stop=(n == NSUB - 1),
                        )
            ctx_sb = outp.tile([1, D], F32, tag="ctx")
            nc.vector.tensor_scalar_mul(ctx_sb[:], ctx_ps[:], invb[0:1, :])
            nc.scalar.dma_start(ctx_d[b], ctx_sb[:])

    nc.compile()
    return nc


def _ensure_ntff_hook():
    """Register the axon NTFF profiling hook if the image's antenv lacks it.

    This image's ``antenv`` package has no ``axon_hooks`` module, so
    ``run_bass_kernel_spmd(trace=True)`` would skip tracing. Recreate the
    module and install the same ctypes-based hook trn_boot would have set.
    """
    import types

    try:
        import antenv.axon_hooks  # noqa: F401
        return
    except ImportError:
        pass
    try:
        import antenv
        from trn_agent_boot.trn_boot import _ntff_profile_via_ctypes
    except ImportError:
        return
    mod = types.ModuleType("antenv.axon_hooks")
    _hook = [None]
    mod.set_axon_ntff_profile_hook = lambda h: _hook.__setitem__(0, h)
    mod.get_axon_ntff_profile_hook = lambda: _hook[0]
    sys.modules["antenv.axon_hooks"] = mod
    antenv.axon_hooks = mod
    so_path = "/opt/axon/libaxon_pjrt.so"
    if os.path.exists(so_path):
        hook = _ntff_profile_via_ctypes(so_path)
        if hook is not None:
            mod.set_axon_ntff_profile_hook(hook)


_NC_CACHE = {}


def _get_nc():
    variant = os.environ.get("ATTN_KERNEL_VARIANT", "f32r")
    chunk = int(os.environ.get("ATTN_KERNEL_CHUNK", "8"))
    key = (variant, chunk)
    if key not in _NC_CACHE:
        if variant == "raw":
            _NC_CACHE[key] = build_nc_raw(chunk=chunk)
        else:
            _NC_CACHE[key] = build_nc(variant=variant, chunk=chunk, debug=False)
    return _NC_CACHE[key]


def kernel(a, h, coverage, X_mask, Wa, Wh, Wc, V, use_coverage, use_masking):
    a = np.asarray(a, dtype=np.float32)
    assert a.shape == (B, TX, D), a.shape
    masking = int(np.asarray(use_masking))
    if masking:
        m = np.asarray(X_mask).reshape(B, TX).astype(np.uint8)
    else:
        m = np.ones((B, TX), dtype=np.uint8)

    variant = os.environ.get("ATTN_KERNEL_VARIANT", "f32r")
    nc = _get_nc()

    def mask_shard(c):
        ms = m[c * BPC:(c + 1) * BPC]  # [BPC, TX] u8
        if variant == "raw":
            # [P, BPC*NSUB] with row p holding mask[b, p*NSUB + n]
            return np.ascontiguousarray(
                ms.reshape(BPC, P, NSUB).transpose(1, 0, 2).reshape(P, BPC * NSUB))
        return np.ascontiguousarray(ms)

    in_maps = [
        {
            "a": np.ascontiguousarray(a[c * BPC:(c + 1) * BPC]),
            "m": mask_shard(c),
        }
        for c in range(NCORES)
    ]
    trace = bool(int(os.environ.get("ATTN_KERNEL_TRACE", "0")))
    if trace:
        _ensure_ntff_hook()
    res = run_bass_kernel_spmd(nc, in_maps, core_ids=list(range(NCORES)),
                               trace=trace)
    if trace:
        kernel.last_exec_time_ns = res.exec_time_ns
        kernel.last_results = res
    context = np.concatenate([r["ctx"] for r in res.results], axis=0)
    alpha = np.concatenate([r["alpha"] for r in res.results], axis=0)
    if not masking:
        # softmax over the size-1 axis gives exactly 1.0 everywhere; without
        # masking there is no normalization, so undo the device-side /count.
        context = context * np.float32(TX)
        alpha = alpha * np.float32(TX)
    return context.reshape(B, 1, D), alpha


# revision 27
# speedup vs baseline: 1.0089x; 1.0089x over previous
"""Trainium2 Bass kernel for nn_Attention_51539608408.

Math note: the reference applies softmax over an axis of size 1, which is
identically 1.0. Consequently the outputs depend only on `a` and `X_mask`:
    alpha[b, t] = mask[b, t] / count[b]          (count = sum_t mask)
    context[b, 0, :] = sum_t mask[b, t] * a[b, t, :] / count[b]
All the Wa/Wh/Wc/V matmuls and tanh are dead compute (verified exactly: the
reference's jax.nn.softmax subtracts the max, so exp(0)/1 == 1.0 exactly).

Sharding: pure data-parallel over batch B=32 -> 4 examples per core x 8 cores.

Per-core device kernel (per example b):
  - t axis (4096) split partition-major: t = p*32 + n  (p in [0,128), n in [0,32))
    so each SBUF partition holds a contiguous 64KB slab of DRAM -> fast DMA.
  - masked sum over t via PE matmul: lhsT = mask column [128,1], rhs = a tile
    [128, 512], accumulated over the 32 sub-tiles into PSUM [1, 512].
  - count via matmul with a ones vector, reciprocal on DVE, broadcast back to
    128 partitions with a tiny [1,128]-ones matmul.
"""

import os
import sys

import numpy as np

for _p in ("/opt/trn_rl_repo", "/opt/trn_rl_repo/concourse"):
    if os.path.isdir(_p) and _p not in sys.path:
        sys.path.insert(0, _p)

from contextlib import ExitStack

from concourse import bacc, bass, mybir, tile
from concourse.bass_utils import run_bass_kernel_spmd

B, TX, D = 32, 4096, 512
NCORES = 8
BPC = B // NCORES  # examples per core
P = 128            # SBUF partitions
NSUB = TX // P     # 32 sub-tiles along t
F32 = mybir.dt.float32
F32R = mybir.dt.float32r
U8 = mybir.dt.uint8


def build_nc_raw(chunk=8, nbuf=8):
    """Hand-scheduled raw-Bass version: no Tile entry/exit barriers.

    Engine roles:
      Sync   — streams the 16 big `a` chunk DMAs back-to-back (HWDGE)
      Scalar — small DMAs: mask in, alpha/ctx out (HWDGE)
      Tensor — 2 batched count matmuls + 4x32 masked-sum f32r matmuls
      Vector — casts, reductions, reciprocal, scaling
      GpSimd — epilogue: restore semaphores to zero for re-executability
    """
    NCHUNK = NSUB // chunk  # chunks per example
    # chunk layout per example: sub-tile counts for each DMA. The final
    # example tapers so the last-DMA -> last-matmul dependency is short.
    chunk_lists = [[chunk] * NCHUNK for _ in range(BPC)]
    chunk_lists[BPC - 1] = [chunk] * (NCHUNK - 1) + [4, 2, 1, 1]
    cum_chunks = []
    tot = 0
    for cl in chunk_lists:
        tot += len(cl)
        cum_chunks.append(tot)
    nc = bass.Bass("TRN2")
    a_d = nc.dram_tensor("a", [BPC, TX, D], F32R, kind="ExternalInput")
    # mask pre-packed on host to [P, BPC*NSUB] so each partition's row is
    # contiguous in DRAM (fast descriptor generation, single DMA)
    m_d = nc.dram_tensor("m", [P, BPC * NSUB], U8, kind="ExternalInput")
    ctx_d = nc.dram_tensor("ctx", [BPC, D], F32, kind="ExternalOutput")
    alpha_d = nc.dram_tensor("alpha", [BPC, TX], F32, kind="ExternalOutput")

    with ExitStack() as st:
        a_bufs = [st.enter_context(nc.sbuf_tensor(f"a_buf{i}", [P, chunk, D], F32R))
                  for i in range(nbuf)]
        m_u8 = st.enter_context(nc.sbuf_tensor("m_u8", [P, BPC, NSUB], U8))
        m_f = st.enter_context(nc.sbuf_tensor("m_f", [P, BPC, NSUB], F32R))
        alpha_sbs = [st.enter_context(nc.sbuf_tensor(f"alpha_sb{b}", [P, NSUB], F32))
                     for b in range(BPC)]
        ctx_sbs = [st.enter_context(nc.sbuf_tensor(f"ctx_sb{b}", [1, D], F32))
                   for b in range(BPC)]
        ones_col = st.enter_context(nc.sbuf_tensor("ones_col", [P, 1], F32))
        ones_row = st.enter_context(nc.sbuf_tensor("ones_row", [1, P], F32))
        partials = st.enter_context(nc.sbuf_tensor("partials", [P, BPC], F32))
        cnt_all = st.enter_context(nc.sbuf_tensor("cnt_all", [1, BPC], F32))
        invb_all = st.enter_context(nc.sbuf_tensor("invb_all", [P, BPC], F32))
        # one full PSUM bank each so PE writes never share a bank with
        # concurrent DVE reads (same-bank PE-W + DVE-R is a hard fault)
        pbank_ctx = [st.enter_context(nc.psum_tensor(f"pb_ctx{b}", [P, D], F32))
                     for b in range(BPC)]
        pbank_cnt = st.enter_context(nc.psum_tensor("pb_cnt", [P, D], F32))
        pbank_cntb = st.enter_context(nc.psum_tensor("pb_cntb", [P, D], F32))
        warm_buf = st.enter_context(nc.sbuf_tensor("warm_buf", [P, D], F32))
        pbank_warm = st.enter_context(nc.psum_tensor("pb_warm", [P, D], F32))

        NCH_ALL = cum_chunks[-1]
        with (
            nc.Block() as block,
            nc.semaphore("sem_m") as sem_m,
            nc.semaphore("sem_dve") as sem_dve,
            nc.semaphore("sem_pe") as sem_pe,
            nc.semaphore("sem_free") as sem_free,
            nc.semaphore("sem_out") as sem_out,
            nc.semaphore("sem_out2") as sem_out2,
            nc.semaphore("sem_warm") as sem_warm,
        ):
            # one semaphore per chunk DMA: HWDGE transfers from one engine can
            # complete out of order across HW queues, so cumulative waits on a
            # single sem are unsound (CoreSim SemaphoreRace).
            sem_a = [st.enter_context(nc.semaphore(f"sem_a{i}"))
                     for i in range(NCH_ALL)]
            sems = [sem_m, sem_dve, sem_pe, sem_free, sem_out, sem_out2,
                    sem_warm] + sem_a
            # DVE tick numbering (sem_dve): cast=1, partials=2..5, cnt=6,
            # recip=7, alpha=8..11, then ctx combines. The last example
            # accumulates into a single PSUM bank so its combine is one op.
            ALPHA_TICK = [8 + b for b in range(BPC)]
            CTX_TICK = []
            t = ALPHA_TICK[-1]
            for b in range(BPC):
                t += 1 if b == BPC - 1 else 2
                CTX_TICK.append(t)

            @block.sync
            def _(sync):
                sync.dma_start(m_u8[:], m_d[:]).then_inc(sem_m, 16)
                i = 0
                for b in range(BPC):
                    a_re = a_d[b].rearrange("(p n) d -> p n d", p=P)
                    off = 0
                    for w in chunk_lists[b]:
                        if i >= nbuf:
                            sync.wait_ge(sem_free, i - nbuf + 1)
                        sync.dma_start(
                            a_bufs[i % nbuf][:, 0:w, :],
                            a_re[:, off:off + w, :],
                        ).then_inc(sem_a[i], 16)
                        off += w
                        i += 1
                for b in range(BPC):
                    sync.wait_ge(sem_dve, CTX_TICK[b])
                    sync.dma_start(ctx_d[b], ctx_sbs[b][0:1, :]).then_inc(
                        sem_out2, 16)



            @block.scalar
            def _(scalar):
                pass  # barrier legs only; all small DMAs ride SWDGE on gpsimd

            @block.vector
            def _(vector):
                nc.vector.memset(warm_buf[:], 1.0).then_inc(sem_warm)
                nc.vector.memset(ones_col[:], 1.0)
                nc.vector.memset(ones_row[:], 1.0)
                vector.wait_ge(sem_m, 16)
                nc.vector.tensor_copy(m_f[:], m_u8[:]).then_inc(sem_dve)  # 1
                vector.wait_ge(sem_dve, 1)  # same-engine RAW on m_f
                for b in range(BPC):
                    nc.vector.reduce_sum(
                        partials[:, b:b + 1], m_f[:, b, :].bitcast(F32),
                        axis=mybir.AxisListType.X,
                    ).then_inc(sem_dve)  # 2+b
                vector.wait_ge(sem_pe, 1)
                nc.vector.tensor_copy(
                    cnt_all[:], pbank_cnt[0:1, 0:BPC]).then_inc(sem_dve)  # 6
                vector.wait_ge(sem_pe, 2)
                nc.vector.reciprocal(
                    invb_all[:], pbank_cntb[:, 0:BPC]).then_inc(sem_dve)  # 7
                vector.wait_ge(sem_dve, 7)  # same-engine RAW on invb_all
                for b in range(BPC):
                    nc.vector.tensor_scalar_mul(
                        alpha_sbs[b][:], m_f[:, b, :].bitcast(F32),
                        invb_all[:, b:b + 1],
                    ).then_inc(sem_dve)  # 8+b
                for b in range(BPC):
                    bankA = pbank_ctx[(b % 2) * 2]
                    vector.wait_ge(sem_free, cum_chunks[b])
                    nc.vector.tensor_scalar_mul(
                        ctx_sbs[b][0:1, :], bankA[0:1, :],
                        invb_all[0:1, b:b + 1],
                    ).then_inc(sem_dve)
                    if b < BPC - 1:
                        bankB = pbank_ctx[(b % 2) * 2 + 1]
                        vector.wait_ge(sem_dve, CTX_TICK[b] - 1)  # same-engine RAW
                        # ctx = (bankB * inv) + ctx  (one PSUM input per DVE op)
                        nc.vector.scalar_tensor_tensor(
                            ctx_sbs[b][0:1, :], bankB[0:1, :],
                            invb_all[0:1, b:b + 1], ctx_sbs[b][0:1, :],
                            op0=mybir.AluOpType.mult, op1=mybir.AluOpType.add,
                        ).then_inc(sem_dve)

            @block.tensor
            def _(tensor):
                # warm the PE HAM window while the first chunks stream in
                tensor.wait_ge(sem_warm, 1)
                for _ in range(3):
                    nc.tensor.matmul(pbank_warm[0:P, :], warm_buf[:, 0:P],
                                     warm_buf[:], start=True, stop=True)

                def chain(tensor, b, i0):
                    # alternate between two PSUM banks so consecutive matmuls
                    # overlap drain with fill (~427ns/mm instead of ~628ns);
                    # the last example uses one bank so the end-of-kernel
                    # combine is a single DVE op
                    dual = b < BPC - 1
                    bankA = pbank_ctx[(b % 2) * 2]
                    bankB = pbank_ctx[(b % 2) * 2 + 1] if dual else bankA
                    i = i0
                    n = 0
                    for w in chunk_lists[b]:
                        tensor.wait_ge(sem_a[i], 16)
                        for j in range(w):
                            mm = nc.tensor.matmul(
                                (bankA if n % 2 == 0 else bankB)[0:1, :],
                                m_f[:, b, n:n + 1],
                                a_bufs[i % nbuf][:, j, :],
                                start=(n <= 1 if dual else n == 0),
                                stop=(n >= NSUB - 2 if dual else n == NSUB - 1),
                            )
                            if j == w - 1:
                                mm.then_inc(sem_free)
                            n += 1
                        i += 1
                    return i

                tensor.wait_ge(sem_dve, 1)  # mask cast done
                i = chain(tensor, 0, 0)
                # counts: after chain 0 so they never delay the pipeline start
                tensor.wait_ge(sem_dve, 1 + BPC)
                nc.tensor.matmul(pbank_cnt[0:1, 0:BPC], ones_col[:], partials[:],
                                 start=True, stop=True).then_inc(sem_pe)
                tensor.wait_ge(sem_dve, 2 + BPC)
                nc.tensor.matmul(pbank_cntb[:, 0:BPC], ones_row[:], cnt_all[:],
                                 start=True, stop=True).then_inc(sem_pe)
                for b in range(1, BPC):
                    if b >= 2:
                        # WAR: chain(b) reuses chain(b-2)'s banks; wait for the
                        # DVE combine of example b-2 to have read BOTH banks
                        tensor.wait_ge(sem_dve, CTX_TICK[b - 2])
                    i = chain(tensor, b, i)

            @block.gpsimd
            def _(gpsimd):
                # outputs ride SWDGE rings, fully separate from the HWDGE
                # queue streaming the 2MB chunk loads
                for b in range(BPC):
                    gpsimd.wait_ge(sem_dve, 8 + b)
                    gpsimd.dma_start(
                        alpha_d[b].rearrange("(p n) -> p n", p=P), alpha_sbs[b][:]
                    ).then_inc(sem_out, 16)
                # hold kernel end until the output DMAs have landed; the
                # all-engine barrier below covers every other semaphore
                gpsimd.wait_ge(sem_out, 16 * BPC)
                gpsimd.wait_ge(sem_out2, 16 * BPC)

            # one all-engine barrier, then restore sems to zero so the NEFF
            # stays correct if the loaded executable is run a second time
            nc.multi_engine_barrier(list(nc.engines))
            nums = sorted(s.num for s in sems)
            assert nums == list(range(nums[0], nums[-1] + 1)), nums
            nc.gpsimd.dma_reset(range(nums[0], nums[-1] + 1))
            nc.gpsimd.sem_clear(range(nums[0], nums[-1] + 1))

    return nc


def build_nc(variant="f32r", chunk=8, debug=False):
    """Build the per-core Bass program.

    variant: 'f32r' (PE matmul, float32r), 'f32' (PE matmul, float32),
             'dve'  (VectorE masked accumulate + one fp32 matmul reduce)
    """
    assert NSUB % chunk == 0
    a_dt = {"f32r": F32R, "f32": F32, "dve": F32}[variant]

    nc = bacc.Bacc("TRN2", target_bir_lowering=False, debug=debug)
    a_d = nc.dram_tensor("a", [BPC, TX, D], a_dt, kind="ExternalInput")
    m_d = nc.dram_tensor("m", [BPC, TX], U8, kind="ExternalInput")
    ctx_d = nc.dram_tensor("ctx", [BPC, D], F32, kind="ExternalOutput")
    alpha_d = nc.dram_tensor("alpha", [BPC, TX], F32, kind="ExternalOutput")

    with tile.TileContext(nc) as tc, ExitStack() as ctx:
        apool = ctx.enter_context(tc.tile_pool(name="a", bufs=5))
        mpool = ctx.enter_context(tc.tile_pool(name="m", bufs=BPC))
        aux = ctx.enter_context(tc.tile_pool(name="aux", bufs=BPC))
        outp = ctx.enter_context(tc.tile_pool(name="out", bufs=2))
        const = ctx.enter_context(tc.tile_pool(name="const", bufs=1))
        pctx = ctx.enter_context(tc.tile_pool(name="pctx", bufs=2, space="PSUM"))
        pcnt = ctx.enter_context(tc.tile_pool(name="pcnt", bufs=2, space="PSUM"))

        ones_col = const.tile([P, 1], F32, tag="ones_col")
        nc.vector.memset(ones_col[:], 1.0)
        ones_row = const.tile([1, P], F32, tag="ones_row")
        nc.vector.memset(ones_row[:], 1.0)

        # Phase 1: everything that depends only on the mask, for all examples.
        # Small DMAs ride the Scalar-engine HWDGE queue so the Sync-engine
        # queue streams the 16 big `a` chunk DMAs with no interleaved waits.
        m_fs, invbs = [], []
        for b in range(BPC):
            m_u8 = mpool.tile([P, NSUB], U8, tag="mu8")
            nc.scalar.dma_start(m_u8[:], m_d[b].rearrange("(p n) -> p n", p=P))
            m_f = mpool.tile([P, NSUB], a_dt, tag="mf")
            nc.vector.tensor_copy(m_f[:], m_u8[:])
            m_fs.append(m_f)

            # count and its reciprocal, broadcast to all partitions
            partials = aux.tile([P, 1], F32, tag="partials")
            nc.vector.reduce_sum(partials[:], m_f[:].bitcast(F32),
                                 axis=mybir.AxisListType.X)
            cnt1_ps = pcnt.tile([1, 1], F32, tag="cnt1")
            nc.tensor.matmul(cnt1_ps[:], ones_col[:], partials[:],
                             start=True, stop=True)
            cnt = aux.tile([1, 1], F32, tag="cnt_sb")
            nc.vector.tensor_copy(cnt[:], cnt1_ps[:])
            cntb_ps = pcnt.tile([P, 1], F32, tag="cntb")
            nc.tensor.matmul(cntb_ps[:], ones_row[:], cnt[:], start=True, stop=True)
            invb = aux.tile([P, 1], F32, tag="invb")
            nc.vector.reciprocal(invb[:], cntb_ps[:])
            invbs.append(invb)

            # alpha = mask * (1/count)
            alpha_sb = outp.tile([P, NSUB], F32, tag="alpha")
            nc.vector.tensor_scalar_mul(alpha_sb[:], m_f[:].bitcast(F32), invb[:])
            nc.scalar.dma_start(alpha_d[b].rearrange("(p n) -> p n", p=P),
                                alpha_sb[:])

        # Phase 2: context = (1/count) * sum_t mask*a — the memory-bound part.
        for b in range(BPC):
            m_f = m_fs[b]
            invb = invbs[b]
            a_re = a_d[b].rearrange("(p n) d -> p n d", p=P)  # [128, 32, 512]
            ctx_ps = pctx.tile([1, D], F32, tag="ctxps")
            if variant == "dve":
                acc = apool.tile([P, D], F32, tag="acc")
                for c in range(NSUB // chunk):
                    a_t = apool.tile([P, chunk, D], a_dt, tag="achunk")
                    nc.sync.dma_start(a_t[:], a_re[:, c * chunk:(c + 1) * chunk, :])
                    for j in range(chunk):
                        n = c * chunk + j
                        if n == 0:
                            nc.vector.tensor_scalar_mul(
                                acc[:], a_t[:, j, :], m_f[:, 0:1])
                        else:
                            masked = apool.tile([P, D], F32, tag="masked")
                            nc.vector.tensor_scalar_mul(
                                masked[:], a_t[:, j, :], m_f[:, n:n + 1])
                            nc.vector.tensor_add(acc[:], acc[:], masked[:])
                nc.tensor.matmul(ctx_ps[:], ones_col[:], acc[:], start=True, stop=True)
            else:
                for c in range(NSUB // chunk):
                    a_t = apool.tile([P, chunk, D], a_dt, tag="achunk")
                    nc.sync.dma_start(a_t[:], a_re[:, c * chunk:(c + 1) * chunk, :])
                    for j in range(chunk):
                        n = c * chunk + j
                        nc.tensor.matmul(
                            ctx_ps[:],
                            m_f[:, n:n + 1],
                            a_t[:, j, :],
                            start=(n == 0),
                            stop=(n == NSUB - 1),
                        )
            ctx_sb = outp.tile([1, D], F32, tag="ctx")
            nc.vector.tensor_scalar_mul(ctx_sb[:], ctx_ps[:], invb[0:1, :])
            nc.scalar.dma_start(ctx_d[b], ctx_sb[:])

    nc.compile()
    return nc


def _ensure_ntff_hook():
    """Register the axon NTFF profiling hook if the image's antenv lacks it.

    This image's ``antenv`` package has no ``axon_hooks`` module, so
    ``run_bass_kernel_spmd(trace=True)`` would skip tracing. Recreate the
    module and install the same ctypes-based hook trn_boot would have set.
    """
    import types

    try:
        import antenv.axon_hooks  # noqa: F401
        return
    except ImportError:
        pass
    try:
        import antenv
        from trn_agent_boot.trn_boot import _ntff_profile_via_ctypes
    except ImportError:
        return
    mod = types.ModuleType("antenv.axon_hooks")
    _hook = [None]
    mod.set_axon_ntff_profile_hook = lambda h: _hook.__setitem__(0, h)
    mod.get_axon_ntff_profile_hook = lambda: _hook[0]
    sys.modules["antenv.axon_hooks"] = mod
    antenv.axon_hooks = mod
    so_path = "/opt/axon/libaxon_pjrt.so"
    if os.path.exists(so_path):
        hook = _ntff_profile_via_ctypes(so_path)
        if hook is not None:
            mod.set_axon_ntff_profile_hook(hook)


_NC_CACHE = {}


def _get_nc():
    variant = os.environ.get("ATTN_KERNEL_VARIANT", "f32r")
    chunk = int(os.environ.get("ATTN_KERNEL_CHUNK", "8"))
    key = (variant, chunk)
    if key not in _NC_CACHE:
        if variant == "raw":
            _NC_CACHE[key] = build_nc_raw(chunk=chunk)
        else:
            _NC_CACHE[key] = build_nc(variant=variant, chunk=chunk, debug=False)
    return _NC_CACHE[key]


def kernel(a, h, coverage, X_mask, Wa, Wh, Wc, V, use_coverage, use_masking):
    a = np.asarray(a, dtype=np.float32)
    assert a.shape == (B, TX, D), a.shape
    masking = int(np.asarray(use_masking))
    if masking:
        m = np.asarray(X_mask).reshape(B, TX).astype(np.uint8)
    else:
        m = np.ones((B, TX), dtype=np.uint8)

    variant = os.environ.get("ATTN_KERNEL_VARIANT", "f32r")
    nc = _get_nc()

    def mask_shard(c):
        ms = m[c * BPC:(c + 1) * BPC]  # [BPC, TX] u8
        if variant == "raw":
            # [P, BPC*NSUB] with row p holding mask[b, p*NSUB + n]
            return np.ascontiguousarray(
                ms.reshape(BPC, P, NSUB).transpose(1, 0, 2).reshape(P, BPC * NSUB))
        return np.ascontiguousarray(ms)

    in_maps = [
        {
            "a": np.ascontiguousarray(a[c * BPC:(c + 1) * BPC]),
            "m": mask_shard(c),
        }
        for c in range(NCORES)
    ]
    trace = bool(int(os.environ.get("ATTN_KERNEL_TRACE", "0")))
    if trace:
        _ensure_ntff_hook()
    res = run_bass_kernel_spmd(nc, in_maps, core_ids=list(range(NCORES)),
                               trace=trace)
    if trace:
        kernel.last_exec_time_ns = res.exec_time_ns
        kernel.last_results = res
    context = np.concatenate([r["ctx"] for r in res.results], axis=0)
    alpha = np.concatenate([r["alpha"] for r in res.results], axis=0)
    if not masking:
        # softmax over the size-1 axis gives exactly 1.0 everywhere; without
        # masking there is no normalization, so undo the device-side /count.
        context = context * np.float32(TX)
        alpha = alpha * np.float32(TX)
    return context.reshape(B, 1, D), alpha


# revision 30
# speedup vs baseline: 1.2213x; 1.2105x over previous
"""Trainium2 Bass kernel for nn_Attention_51539608408.

Math note: the reference applies softmax over an axis of size 1, which is
identically 1.0. Consequently the outputs depend only on `a` and `X_mask`:
    alpha[b, t] = mask[b, t] / count[b]          (count = sum_t mask)
    context[b, 0, :] = sum_t mask[b, t] * a[b, t, :] / count[b]
All the Wa/Wh/Wc/V matmuls and tanh are dead compute (verified exactly: the
reference's jax.nn.softmax subtracts the max, so exp(0)/1 == 1.0 exactly).

Sharding: pure data-parallel over batch B=32 -> 4 examples per core x 8 cores.

Per-core device kernel (per example b):
  - t axis (4096) split partition-major: t = p*32 + n  (p in [0,128), n in [0,32))
    so each SBUF partition holds a contiguous 64KB slab of DRAM -> fast DMA.
  - masked sum over t via PE matmul: lhsT = mask column [128,1], rhs = a tile
    [128, 512], accumulated over the 32 sub-tiles into PSUM [1, 512].
  - count via matmul with a ones vector, reciprocal on DVE, broadcast back to
    128 partitions with a tiny [1,128]-ones matmul.
"""

import os
import sys

import numpy as np

for _p in ("/opt/trn_rl_repo", "/opt/trn_rl_repo/concourse"):
    if os.path.isdir(_p) and _p not in sys.path:
        sys.path.insert(0, _p)

from contextlib import ExitStack

from concourse import bacc, bass, mybir, tile
from concourse.bass_utils import run_bass_kernel_spmd

B, TX, D = 32, 4096, 512
NCORES = 8
BPC = B // NCORES  # examples per core
P = 128            # SBUF partitions
NSUB = TX // P     # 32 sub-tiles along t
F32 = mybir.dt.float32
F32R = mybir.dt.float32r
U8 = mybir.dt.uint8


def build_nc_raw(chunk=8, nbuf=8):
    """Hand-scheduled raw-Bass version: no Tile entry/exit barriers.

    Engine roles:
      Sync   — streams the 16 big `a` chunk DMAs back-to-back (HWDGE)
      Scalar — small DMAs: mask in, alpha/ctx out (HWDGE)
      Tensor — 2 batched count matmuls + 4x32 masked-sum f32r matmuls
      Vector — casts, reductions, reciprocal, scaling
      GpSimd — epilogue: restore semaphores to zero for re-executability
    """
    NCHUNK = NSUB // chunk  # chunks per example
    # chunk layout per example: sub-tile counts for each DMA. The final
    # example tapers so the last-DMA -> last-matmul dependency is short.
    chunk_lists = [[chunk] * NCHUNK for _ in range(BPC)]
    taper = []
    w = chunk // 2
    while w >= 1:
        taper.append(w)
        w //= 2
    taper.append(1)  # e.g. chunk=8 -> [4, 2, 1, 1]
    chunk_lists[BPC - 1] = [chunk] * (NCHUNK - 1) + taper
    cum_chunks = []
    tot = 0
    for cl in chunk_lists:
        tot += len(cl)
        cum_chunks.append(tot)
    nc = bass.Bass("TRN2")
    a_d = nc.dram_tensor("a", [BPC, TX, D], F32R, kind="ExternalInput")
    # mask pre-packed on host to [P, BPC*NSUB] so each partition's row is
    # contiguous in DRAM (fast descriptor generation, single DMA)
    m_d = nc.dram_tensor("m", [P, BPC * NSUB], U8, kind="ExternalInput")
    ctx_d = nc.dram_tensor("ctx", [BPC, D], F32, kind="ExternalOutput")
    alpha_d = nc.dram_tensor("alpha", [BPC, TX], F32, kind="ExternalOutput")

    with ExitStack() as st:
        a_bufs = [st.enter_context(nc.sbuf_tensor(f"a_buf{i}", [P, chunk, D], F32R))
                  for i in range(nbuf)]
        m_u8 = st.enter_context(nc.sbuf_tensor("m_u8", [P, BPC, NSUB], U8))
        m_f = st.enter_context(nc.sbuf_tensor("m_f", [P, BPC, NSUB], F32R))
        alpha_sbs = [st.enter_context(nc.sbuf_tensor(f"alpha_sb{b}", [P, NSUB], F32))
                     for b in range(BPC)]
        ctx_sbs = [st.enter_context(nc.sbuf_tensor(f"ctx_sb{b}", [1, D], F32))
                   for b in range(BPC)]
        ones_col = st.enter_context(nc.sbuf_tensor("ones_col", [P, 1], F32))
        ones_row = st.enter_context(nc.sbuf_tensor("ones_row", [1, P], F32))
        partials = st.enter_context(nc.sbuf_tensor("partials", [P, BPC], F32))
        cnt_all = st.enter_context(nc.sbuf_tensor("cnt_all", [1, BPC], F32))
        invb_all = st.enter_context(nc.sbuf_tensor("invb_all", [P, BPC], F32))
        # one full PSUM bank each so PE writes never share a bank with
        # concurrent DVE reads (same-bank PE-W + DVE-R is a hard fault)
        pbank_ctx = [st.enter_context(nc.psum_tensor(f"pb_ctx{b}", [P, D], F32))
                     for b in range(BPC)]
        pbank_cnt = st.enter_context(nc.psum_tensor("pb_cnt", [P, D], F32))
        pbank_cntb = st.enter_context(nc.psum_tensor("pb_cntb", [P, D], F32))
        warm_buf = st.enter_context(nc.sbuf_tensor("warm_buf", [P, D], F32))
        pbank_warm = st.enter_context(nc.psum_tensor("pb_warm", [P, D], F32))

        NCH_ALL = cum_chunks[-1]
        with (
            nc.Block() as block,
            nc.semaphore("sem_m") as sem_m,
            nc.semaphore("sem_dve") as sem_dve,
            nc.semaphore("sem_pe") as sem_pe,
            nc.semaphore("sem_free") as sem_free,
            nc.semaphore("sem_out") as sem_out,
            nc.semaphore("sem_out2") as sem_out2,
            nc.semaphore("sem_warm") as sem_warm,
        ):
            # one semaphore per chunk DMA: HWDGE transfers from one engine can
            # complete out of order across HW queues, so cumulative waits on a
            # single sem are unsound (CoreSim SemaphoreRace).
            sem_a = [st.enter_context(nc.semaphore(f"sem_a{i}"))
                     for i in range(NCH_ALL)]
            sems = [sem_m, sem_dve, sem_pe, sem_free, sem_out, sem_out2,
                    sem_warm] + sem_a
            # DVE tick numbering (sem_dve): cast=1, partials=2..5, cnt=6,
            # recip=7, alpha=8..11, then ctx combines. The last example
            # accumulates into a single PSUM bank so its combine is one op.
            ALPHA_TICK = [8 + b for b in range(BPC)]
            CTX_TICK = []
            t = ALPHA_TICK[-1]
            for b in range(BPC):
                t += 1 if b == BPC - 1 else 2
                CTX_TICK.append(t)

            @block.sync
            def _(sync):
                sync.dma_start(m_u8[:], m_d[:]).then_inc(sem_m, 16)
                i = 0
                for b in range(BPC):
                    a_re = a_d[b].rearrange("(p n) d -> p n d", p=P)
                    off = 0
                    for w in chunk_lists[b]:
                        if i >= nbuf:
                            sync.wait_ge(sem_free, i - nbuf + 1)
                        sync.dma_start(
                            a_bufs[i % nbuf][:, 0:w, :],
                            a_re[:, off:off + w, :],
                        ).then_inc(sem_a[i], 16)
                        off += w
                        i += 1
                for b in range(BPC):
                    sync.wait_ge(sem_dve, CTX_TICK[b])
                    sync.dma_start(ctx_d[b], ctx_sbs[b][0:1, :]).then_inc(
                        sem_out2, 16)



            @block.scalar
            def _(scalar):
                pass  # barrier legs only; all small DMAs ride SWDGE on gpsimd

            @block.vector
            def _(vector):
                nc.vector.memset(warm_buf[:], 1.0).then_inc(sem_warm)
                nc.vector.memset(ones_col[:], 1.0)
                nc.vector.memset(ones_row[:], 1.0)
                vector.wait_ge(sem_m, 16)
                nc.vector.tensor_copy(m_f[:], m_u8[:]).then_inc(sem_dve)  # 1
                vector.wait_ge(sem_dve, 1)  # same-engine RAW on m_f
                for b in range(BPC):
                    nc.vector.reduce_sum(
                        partials[:, b:b + 1], m_f[:, b, :].bitcast(F32),
                        axis=mybir.AxisListType.X,
                    ).then_inc(sem_dve)  # 2+b
                vector.wait_ge(sem_pe, 1)
                nc.vector.tensor_copy(
                    cnt_all[:], pbank_cnt[0:1, 0:BPC]).then_inc(sem_dve)  # 6
                vector.wait_ge(sem_pe, 2)
                nc.vector.reciprocal(
                    invb_all[:], pbank_cntb[:, 0:BPC]).then_inc(sem_dve)  # 7
                vector.wait_ge(sem_dve, 7)  # same-engine RAW on invb_all
                for b in range(BPC):
                    nc.vector.tensor_scalar_mul(
                        alpha_sbs[b][:], m_f[:, b, :].bitcast(F32),
                        invb_all[:, b:b + 1],
                    ).then_inc(sem_dve)  # 8+b
                for b in range(BPC):
                    bankA = pbank_ctx[(b % 2) * 2]
                    vector.wait_ge(sem_free, cum_chunks[b])
                    nc.vector.tensor_scalar_mul(
                        ctx_sbs[b][0:1, :], bankA[0:1, :],
                        invb_all[0:1, b:b + 1],
                    ).then_inc(sem_dve)
                    if b < BPC - 1:
                        bankB = pbank_ctx[(b % 2) * 2 + 1]
                        vector.wait_ge(sem_dve, CTX_TICK[b] - 1)  # same-engine RAW
                        # ctx = (bankB * inv) + ctx  (one PSUM input per DVE op)
                        nc.vector.scalar_tensor_tensor(
                            ctx_sbs[b][0:1, :], bankB[0:1, :],
                            invb_all[0:1, b:b + 1], ctx_sbs[b][0:1, :],
                            op0=mybir.AluOpType.mult, op1=mybir.AluOpType.add,
                        ).then_inc(sem_dve)

            @block.tensor
            def _(tensor):
                # warm the PE HAM window while the first chunks stream in
                tensor.wait_ge(sem_warm, 1)
                for _ in range(3):
                    nc.tensor.matmul(pbank_warm[0:P, :], warm_buf[:, 0:P],
                                     warm_buf[:], start=True, stop=True)

                def chain(tensor, b, i0):
                    # alternate between two PSUM banks so consecutive matmuls
                    # overlap drain with fill (~427ns/mm instead of ~628ns);
                    # the last example uses one bank so the end-of-kernel
                    # combine is a single DVE op
                    dual = b < BPC - 1
                    bankA = pbank_ctx[(b % 2) * 2]
                    bankB = pbank_ctx[(b % 2) * 2 + 1] if dual else bankA
                    i = i0
                    n = 0
                    for w in chunk_lists[b]:
                        tensor.wait_ge(sem_a[i], 16)
                        for j in range(w):
                            mm = nc.tensor.matmul(
                                (bankA if n % 2 == 0 else bankB)[0:1, :],
                                m_f[:, b, n:n + 1],
                                a_bufs[i % nbuf][:, j, :],
                                start=(n <= 1 if dual else n == 0),
                                stop=(n >= NSUB - 2 if dual else n == NSUB - 1),
                            )
                            if j == w - 1:
                                mm.then_inc(sem_free)
                            n += 1
                        i += 1
                    return i

                tensor.wait_ge(sem_dve, 1)  # mask cast done
                i = chain(tensor, 0, 0)
                # counts: after chain 0 so they never delay the pipeline start
                tensor.wait_ge(sem_dve, 1 + BPC)
                nc.tensor.matmul(pbank_cnt[0:1, 0:BPC], ones_col[:], partials[:],
                                 start=True, stop=True).then_inc(sem_pe)
                tensor.wait_ge(sem_dve, 2 + BPC)
                nc.tensor.matmul(pbank_cntb[:, 0:BPC], ones_row[:], cnt_all[:],
                                 start=True, stop=True).then_inc(sem_pe)
                for b in range(1, BPC):
                    if b >= 2:
                        # WAR: chain(b) reuses chain(b-2)'s banks; wait for the
                        # DVE combine of example b-2 to have read BOTH banks
                        tensor.wait_ge(sem_dve, CTX_TICK[b - 2])
                    i = chain(tensor, b, i)

            @block.gpsimd
            def _(gpsimd):
                # outputs ride SWDGE rings, fully separate from the HWDGE
                # queue streaming the 2MB chunk loads
                for b in range(BPC):
                    gpsimd.wait_ge(sem_dve, 8 + b)
                    gpsimd.dma_start(
                        alpha_d[b].rearrange("(p n) -> p n", p=P), alpha_sbs[b][:]
                    ).then_inc(sem_out, 16)
                # hold kernel end until the output DMAs have landed; the
                # all-engine barrier below covers every other semaphore
                gpsimd.wait_ge(sem_out, 16 * BPC)
                gpsimd.wait_ge(sem_out2, 16 * BPC)

            # one all-engine barrier, then restore sems to zero so the NEFF
            # stays correct if the loaded executable is run a second time
            nc.multi_engine_barrier(list(nc.engines))
            nums = sorted(s.num for s in sems)
            assert nums == list(range(nums[0], nums[-1] + 1)), nums
            nc.gpsimd.dma_reset(range(nums[0], nums[-1] + 1))
            nc.gpsimd.sem_clear(range(nums[0], nums[-1] + 1))

    return nc


def build_nc(variant="f32r", chunk=8, debug=False):
    """Build the per-core Bass program.

    variant: 'f32r' (PE matmul, float32r), 'f32' (PE matmul, float32),
             'dve'  (VectorE masked accumulate + one fp32 matmul reduce)
    """
    assert NSUB % chunk == 0
    a_dt = {"f32r": F32R, "f32": F32, "dve": F32}[variant]

    nc = bacc.Bacc("TRN2", target_bir_lowering=False, debug=debug)
    a_d = nc.dram_tensor("a", [BPC, TX, D], a_dt, kind="ExternalInput")
    m_d = nc.dram_tensor("m", [BPC, TX], U8, kind="ExternalInput")
    ctx_d = nc.dram_tensor("ctx", [BPC, D], F32, kind="ExternalOutput")
    alpha_d = nc.dram_tensor("alpha", [BPC, TX], F32, kind="ExternalOutput")

    with tile.TileContext(nc) as tc, ExitStack() as ctx:
        apool = ctx.enter_context(tc.tile_pool(name="a", bufs=5))
        mpool = ctx.enter_context(tc.tile_pool(name="m", bufs=BPC))
        aux = ctx.enter_context(tc.tile_pool(name="aux", bufs=BPC))
        outp = ctx.enter_context(tc.tile_pool(name="out", bufs=2))
        const = ctx.enter_context(tc.tile_pool(name="const", bufs=1))
        pctx = ctx.enter_context(tc.tile_pool(name="pctx", bufs=2, space="PSUM"))
        pcnt = ctx.enter_context(tc.tile_pool(name="pcnt", bufs=2, space="PSUM"))

        ones_col = const.tile([P, 1], F32, tag="ones_col")
        nc.vector.memset(ones_col[:], 1.0)
        ones_row = const.tile([1, P], F32, tag="ones_row")
        nc.vector.memset(ones_row[:], 1.0)

        # Phase 1: everything that depends only on the mask, for all examples.
        # Small DMAs ride the Scalar-engine HWDGE queue so the Sync-engine
        # queue streams the 16 big `a` chunk DMAs with no interleaved waits.
        m_fs, invbs = [], []
        for b in range(BPC):
            m_u8 = mpool.tile([P, NSUB], U8, tag="mu8")
            nc.scalar.dma_start(m_u8[:], m_d[b].rearrange("(p n) -> p n", p=P))
            m_f = mpool.tile([P, NSUB], a_dt, tag="mf")
            nc.vector.tensor_copy(m_f[:], m_u8[:])
            m_fs.append(m_f)

            # count and its reciprocal, broadcast to all partitions
            partials = aux.tile([P, 1], F32, tag="partials")
            nc.vector.reduce_sum(partials[:], m_f[:].bitcast(F32),
                                 axis=mybir.AxisListType.X)
            cnt1_ps = pcnt.tile([1, 1], F32, tag="cnt1")
            nc.tensor.matmul(cnt1_ps[:], ones_col[:], partials[:],
                             start=True, stop=True)
            cnt = aux.tile([1, 1], F32, tag="cnt_sb")
            nc.vector.tensor_copy(cnt[:], cnt1_ps[:])
            cntb_ps = pcnt.tile([P, 1], F32, tag="cntb")
            nc.tensor.matmul(cntb_ps[:], ones_row[:], cnt[:], start=True, stop=True)
            invb = aux.tile([P, 1], F32, tag="invb")
            nc.vector.reciprocal(invb[:], cntb_ps[:])
            invbs.append(invb)

            # alpha = mask * (1/count)
            alpha_sb = outp.tile([P, NSUB], F32, tag="alpha")
            nc.vector.tensor_scalar_mul(alpha_sb[:], m_f[:].bitcast(F32), invb[:])
            nc.scalar.dma_start(alpha_d[b].rearrange("(p n) -> p n", p=P),
                                alpha_sb[:])

        # Phase 2: context = (1/count) * sum_t mask*a — the memory-bound part.
        for b in range(BPC):
            m_f = m_fs[b]
            invb = invbs[b]
            a_re = a_d[b].rearrange("(p n) d -> p n d", p=P)  # [128, 32, 512]
            ctx_ps = pctx.tile([1, D], F32, tag="ctxps")
            if variant == "dve":
                acc = apool.tile([P, D], F32, tag="acc")
                for c in range(NSUB // chunk):
                    a_t = apool.tile([P, chunk, D], a_dt, tag="achunk")
                    nc.sync.dma_start(a_t[:], a_re[:, c * chunk:(c + 1) * chunk, :])
                    for j in range(chunk):
                        n = c * chunk + j
                        if n == 0:
                            nc.vector.tensor_scalar_mul(
                                acc[:], a_t[:, j, :], m_f[:, 0:1])
                        else:
                            masked = apool.tile([P, D], F32, tag="masked")
                            nc.vector.tensor_scalar_mul(
                                masked[:], a_t[:, j, :], m_f[:, n:n + 1])
                            nc.vector.tensor_add(acc[:], acc[:], masked[:])
                nc.tensor.matmul(ctx_ps[:], ones_col[:], acc[:], start=True, stop=True)
            else:
                for c in range(NSUB // chunk):
                    a_t = apool.tile([P, chunk, D], a_dt, tag="achunk")
                    nc.sync.dma_start(a_t[:], a_re[:, c * chunk:(c + 1) * chunk, :])
                    for j in range(chunk):
                        n = c * chunk + j
                        nc.tensor.matmul(
                            ctx_ps[:],
                            m_f[:, n:n + 1],
                            a_t[:, j, :],
                            start=(n == 0),
                            stop=(n == NSUB - 1),
                        )
            ctx_sb = outp.tile([1, D], F32, tag="ctx")
            nc.vector.tensor_scalar_mul(ctx_sb[:], ctx_ps[:], invb[0:1, :])
            nc.scalar.dma_start(ctx_d[b], ctx_sb[:])

    nc.compile()
    return nc


def _ensure_ntff_hook():
    """Register the axon NTFF profiling hook if the image's antenv lacks it.

    This image's ``antenv`` package has no ``axon_hooks`` module, so
    ``run_bass_kernel_spmd(trace=True)`` would skip tracing. Recreate the
    module and install the same ctypes-based hook trn_boot would have set.
    """
    import types

    try:
        import antenv.axon_hooks  # noqa: F401
        return
    except ImportError:
        pass
    try:
        import antenv
        from trn_agent_boot.trn_boot import _ntff_profile_via_ctypes
    except ImportError:
        return
    mod = types.ModuleType("antenv.axon_hooks")
    _hook = [None]
    mod.set_axon_ntff_profile_hook = lambda h: _hook.__setitem__(0, h)
    mod.get_axon_ntff_profile_hook = lambda: _hook[0]
    sys.modules["antenv.axon_hooks"] = mod
    antenv.axon_hooks = mod
    so_path = "/opt/axon/libaxon_pjrt.so"
    if os.path.exists(so_path):
        hook = _ntff_profile_via_ctypes(so_path)
        if hook is not None:
            mod.set_axon_ntff_profile_hook(hook)


_NC_CACHE = {}


def _get_nc():
    variant = os.environ.get("ATTN_KERNEL_VARIANT", "raw")
    chunk = int(os.environ.get("ATTN_KERNEL_CHUNK", "8"))
    nbuf = int(os.environ.get("ATTN_KERNEL_NBUF", "8"))
    key = (variant, chunk, nbuf)
    if key not in _NC_CACHE:
        if variant == "raw":
            _NC_CACHE[key] = build_nc_raw(chunk=chunk, nbuf=nbuf)
        else:
            _NC_CACHE[key] = build_nc(variant=variant, chunk=chunk, debug=False)
    return _NC_CACHE[key]


def kernel(a, h, coverage, X_mask, Wa, Wh, Wc, V, use_coverage, use_masking):
    a = np.asarray(a, dtype=np.float32)
    assert a.shape == (B, TX, D), a.shape
    masking = int(np.asarray(use_masking))
    if masking:
        m = np.asarray(X_mask).reshape(B, TX).astype(np.uint8)
    else:
        m = np.ones((B, TX), dtype=np.uint8)

    variant = os.environ.get("ATTN_KERNEL_VARIANT", "raw")
    nc = _get_nc()

    def mask_shard(c):
        ms = m[c * BPC:(c + 1) * BPC]  # [BPC, TX] u8
        if variant == "raw":
            # [P, BPC*NSUB] with row p holding mask[b, p*NSUB + n]
            return np.ascontiguousarray(
                ms.reshape(BPC, P, NSUB).transpose(1, 0, 2).reshape(P, BPC * NSUB))
        return np.ascontiguousarray(ms)

    in_maps = [
        {
            "a": np.ascontiguousarray(a[c * BPC:(c + 1) * BPC]),
            "m": mask_shard(c),
        }
        for c in range(NCORES)
    ]
    trace = bool(int(os.environ.get("ATTN_KERNEL_TRACE", "0")))
    if trace:
        _ensure_ntff_hook()
    res = run_bass_kernel_spmd(nc, in_maps, core_ids=list(range(NCORES)),
                               trace=trace)
    if trace:
        kernel.last_exec_time_ns = res.exec_time_ns
        kernel.last_results = res
    context = np.concatenate([r["ctx"] for r in res.results], axis=0)
    alpha = np.concatenate([r["alpha"] for r in res.results], axis=0)
    if not masking:
        # softmax over the size-1 axis gives exactly 1.0 everywhere; without
        # masking there is no normalization, so undo the device-side /count.
        context = context * np.float32(TX)
        alpha = alpha * np.float32(TX)
    return context.reshape(B, 1, D), alpha
